# revision 10
# baseline (speedup 1.0000x reference)
"""GATv2 AttentionEncoder kernel for Trainium2 (8 NeuronCores, Bass/Tile).

Strategy (sharding_hint: shard by graph):
  - 256 graphs -> 8 cores x 32 graphs. Each core owns a contiguous,
    graph-aligned node slice, padded to NMAXP rows (multiple of 128).
  - Per layer: node-phase matmuls (xl = h@Wl+bl, xr = h@Wr+br) run on the
    local slice in bf16; xl is AllGathered (src edges reference any node),
    xr stays resident in SBUF (edges are bucketed by dst core/range).
  - Edge phase: edges sorted by dst 128-node range. Host precomputes, per
    128-edge tile, the dst one-hot W1 [edge, dst] and its transpose W1T
    [dst, edge] (bf16, zero rows for padding slots). Per range: dma_gather
    xl rows (int16 idx, lo/hi window split for the 32k limit); xr rows come
    from a PE matmul with lhsT=W1T (no SWDGE gather); all per-edge
    elementwise work (leaky_relu, att-dot, exp, alpha-scaling) runs as
    whole-range batched DVE/ACT ops; the scatter is a PE matmul with
    lhsT=W1 and rhs=[alpha-scaled xl | exp(logit)] producing the weighted
    sum and the softmax denominator in one accumulation group.
  - Pooling is a host-precomputed one-hot matmul + 1/cnt post-scale; the
    MLP head runs per-core on its 32 graphs in fp32; host concatenates.
"""

import sys

sys.path.insert(0, "/opt/trn_rl_repo")

import contextlib

import ml_dtypes
import numpy as np

import concourse.bass as bass
import concourse.bacc as bacc
import concourse.mybir as mybir
import concourse.tile as tile
from concourse.bass_utils import run_bass_kernel_spmd

F32 = mybir.dt.float32
BF16 = mybir.dt.bfloat16
I16 = mybir.dt.int16
AX = mybir.AxisListType
OP = mybir.AluOpType
ACTF = mybir.ActivationFunctionType

SLOPE = 0.2
BF = ml_dtypes.bfloat16


# ----------------------------------------------------------------------------
# Host-side preprocessing
# ----------------------------------------------------------------------------

def _wrap_idx(arr):
    """[n] int array (n % 16 == 0) -> [128, n/16] int16, slot i at
    [i%16, i//16], replicated 8x across partition groups of 16."""
    n = len(arr)
    w = np.ascontiguousarray(arr.reshape(n // 16, 16).T).astype(np.int16)
    return np.tile(w, (8, 1))


def preprocess(inputs, n_cores=8, split=32768):
    x = np.asarray(inputs["x"], np.float32)
    ei = np.asarray(inputs["edge_index"], np.int64)
    batch = np.asarray(inputs["batch"], np.int64)
    N, n_in = x.shape
    G = inputs["_n_graphs"]
    gpc = G // n_cores
    dims = inputs["_dims"]
    ebytes = 2
    # row byte-stride of xl must be a multiple of 256B
    cpads = [(co * ebytes + 255) // 256 * 256 // ebytes for (_, co) in dims]

    cnt = np.bincount(batch, minlength=G)
    gs = np.add.reduceat(cnt, np.arange(0, G, gpc))  # nodes per core
    bounds = np.concatenate([[0], np.cumsum(gs)]).astype(np.int64)
    NMAXP = int((gs.max() + 127) // 128 * 128)
    R = NMAXP // 128
    NFULL = n_cores * NMAXP
    assert NFULL <= split * 2, (NFULL, split)

    # remap node ids into the padded global layout
    newid = np.empty(N, np.int64)
    for r in range(n_cores):
        n0, n1 = bounds[r], bounds[r + 1]
        newid[n0:n1] = NMAXP * r + np.arange(n1 - n0)

    loops = np.arange(N, dtype=np.int64)
    src = newid[np.concatenate([ei[0], loops])]
    dst = newid[np.concatenate([ei[1], loops])]

    # bucket edges per (core, range, lo/hi); order within a bucket irrelevant
    core_of = dst // NMAXP
    dstl = dst - core_of * NMAXP
    rng_of = dstl // 128
    is_hi = (src >= split).astype(np.int64)
    key = (core_of * R + rng_of) * 2 + is_hi
    order = np.argsort(key, kind="stable")
    src_s, dstl_s, key_s = src[order], dstl[order], key[order]
    uniq, starts = np.unique(key_s, return_index=True)
    starts = list(starts) + [len(key_s)]
    lo_lists = [[None] * R for _ in range(n_cores)]
    hi_lists = [[None] * R for _ in range(n_cores)]
    for i, k in enumerate(uniq):
        e0, e1 = starts[i], starts[i + 1]
        c, rem = divmod(int(k), 2 * R)
        g, h = divmod(rem, 2)
        pair = (src_s[e0:e1], dstl_s[e0:e1])
        (hi_lists if h else lo_lists)[c][g] = pair

    empty = (np.zeros(0, np.int64), np.zeros(0, np.int64))
    TLO = np.zeros(R, np.int64)
    THI = np.zeros(R, np.int64)
    for g in range(R):
        for c in range(n_cores):
            lo = lo_lists[c][g] or empty
            hi = hi_lists[c][g] or empty
            TLO[g] = max(TLO[g], -(-len(lo[0]) // 128))
            THI[g] = max(THI[g], -(-len(hi[0]) // 128))
    T = TLO + THI
    sumT = int(T.sum())

    meta = dict(
        n_cores=n_cores, gpc=gpc, G=G, NMAXP=NMAXP, R=R, NFULL=NFULL,
        split=split, TLO=TLO.tolist(), THI=THI.tolist(), T=T.tolist(),
        dims=dims, cpads=cpads, n_in=n_in, sumT=sumT,
        nhid=inputs["_nhid"], nout=inputs["_nout"],
    )

    # ---- shared const arrays ----
    ident = np.eye(128, dtype=np.float32)

    def bc(v, w, dt=np.float32):  # broadcast a [w] vector to [128, w]
        return np.tile(np.asarray(v, dt).reshape(1, w), (128, 1))

    def padk(w, dt=np.float32):  # pad leading dim to a multiple of 128
        k = (-(-w.shape[0] // 128)) * 128
        out = np.zeros((k,) + w.shape[1:], dt)
        out[: w.shape[0]] = np.asarray(w, dt)
        return out

    consts = dict(ident=ident)
    for l, (ci, co) in enumerate(dims):
        consts[f"wl{l}"] = padk(inputs[f"Wl{l}"], BF)
        consts[f"wr{l}"] = padk(inputs[f"Wr{l}"], BF)
        bl = np.asarray(inputs[f"bl{l}"], np.float64)
        consts[f"brb{l}"] = bc(np.asarray(inputs[f"br{l}"], np.float64) + bl, co)
        consts[f"bib{l}"] = bc(np.asarray(inputs[f"bias{l}"], np.float64) + bl, co)
        consts[f"attb{l}"] = bc(inputs[f"att{l}"], co, BF)
    consts["fc1"] = padk(inputs["fc1_W"])
    consts["fc2"] = padk(inputs["fc2_W"])
    consts["b1b"] = bc(inputs["fc1_b"], meta["nhid"])
    consts["b2b"] = bc(inputs["fc2_b"], meta["nout"])

    rcnt = 1.0 / np.maximum(cnt, 1).astype(np.float64)
    KIN = -(-n_in // 128)

    in_maps = []
    for c in range(n_cores):
        n0, n1 = bounds[c], bounds[c + 1]
        nl = int(n1 - n0)
        xT = np.zeros((KIN * 128, NMAXP), BF)
        xT[:n_in, :nl] = x[n0:n1].T.astype(BF)
        # per-core one-hot tiles + gather idxs
        w1 = np.zeros((128, sumT * 128), BF)
        w1t = np.zeros((128, sumT * 128), BF)
        ilo, ihi = [], []
        toff = 0
        for g in range(R):
            lo = lo_lists[c][g] or empty
            hi = hi_lists[c][g] or empty
            nlo, nhi = 128 * int(TLO[g]), 128 * int(THI[g])
            sl = np.zeros(nlo, np.int64)
            sl[: len(lo[0])] = lo[0]
            sh = np.zeros(nhi, np.int64)
            sh[: len(hi[0])] = hi[0] - split
            if nlo:
                ilo.append(_wrap_idx(sl))
            if nhi:
                ihi.append(_wrap_idx(sh))
            # real slots: dst one-hots (pad slots stay all-zero)
            dd = np.concatenate([lo[1], hi[1] + 0]).astype(np.int64)
            slot = np.concatenate(
                [np.arange(len(lo[1])), nlo + np.arange(len(hi[1]))])
            j = dd - g * 128  # local dst within range, 0..127
            p = slot % 128
            t = toff + slot // 128
            w1[p, t * 128 + j] = 1.0
            w1t[j, t * 128 + p] = 1.0
            toff += int(T[g])
        # pooling one-hot [node-in-range, graph-slot] per range, bf16 exact
        wp = np.zeros((128, R * 128), BF)
        bloc = (batch[n0:n1] - c * gpc).astype(np.int64)
        nodes = np.arange(nl)
        wp[nodes % 128, (nodes // 128) * 128 + bloc] = 1.0
        rc = np.zeros(128, np.float32)
        rc[:gpc] = rcnt[c * gpc:(c + 1) * gpc]
        m = dict(
            xT=xT, w1=w1, w1t=w1t, wp=wp, rcg=rc.reshape(128, 1),
            idx_lo=np.concatenate(ilo, 1) if ilo else np.zeros((128, 1), np.int16),
            idx_hi=np.concatenate(ihi, 1) if ihi else np.zeros((128, 1), np.int16),
        )
        m.update(consts)
        in_maps.append({k: np.ascontiguousarray(v) for k, v in m.items()})

    return meta, in_maps


# ----------------------------------------------------------------------------
# Bass program
# ----------------------------------------------------------------------------

def build_nc(meta):
    n_cores = meta["n_cores"]
    NMAXP, R, NFULL = meta["NMAXP"], meta["R"], meta["NFULL"]
    split = meta["split"]
    TLO, THI, T = meta["TLO"], meta["THI"], meta["T"]
    dims, cpads = meta["dims"], meta["cpads"]
    n_in, nhid, nout, gpc = meta["n_in"], meta["nhid"], meta["nout"], meta["gpc"]
    sumT = meta["sumT"]
    KIN = -(-n_in // 128)
    KH = -(-nhid // 128)
    n_layers = len(dims)
    Tmax = max(T)
    CPmax = max(cpads)
    COmax = max(co for _, co in dims)
    rg = [list(range(n_cores))]

    nc = bacc.Bacc(trn_type="TRN2", num_devices=n_cores)

    def inp(name, shape, dtype=F32):
        return nc.dram_tensor(name, list(shape), dtype, kind="ExternalInput").ap()

    xT = inp("xT", [KIN * 128, NMAXP], BF16)
    idx_lo = inp("idx_lo", [128, max(8 * sum(TLO), 1)], I16)
    idx_hi = inp("idx_hi", [128, max(8 * sum(THI), 1)], I16)
    w1_i = inp("w1", [128, sumT * 128], BF16)
    w1t_i = inp("w1t", [128, sumT * 128], BF16)
    wp_i = inp("wp", [128, R * 128], BF16)
    rcg_i = inp("rcg", [128, 1])
    ident_i = inp("ident", [128, 128])
    w_i = {}
    for l, (ci, co) in enumerate(dims):
        kc = -(-ci // 128)
        w_i[f"wl{l}"] = inp(f"wl{l}", [kc * 128, co], BF16)
        w_i[f"wr{l}"] = inp(f"wr{l}", [kc * 128, co], BF16)
        for nm in ("brb", "bib"):
            w_i[f"{nm}{l}"] = inp(f"{nm}{l}", [128, co])
        w_i[f"attb{l}"] = inp(f"attb{l}", [128, co], BF16)
    fc1_i = inp("fc1", [KH * 128, nhid])
    fc2_i = inp("fc2", [KH * 128, nout])
    b1b_i = inp("b1b", [128, nhid])
    b2b_i = inp("b2b", [128, nout])
    out_t = nc.dram_tensor("out", [gpc, nout], F32, kind="ExternalOutput").ap()

    with tile.TileContext(nc) as tc, contextlib.ExitStack() as ctx:
        cpool = ctx.enter_context(tc.tile_pool(name="consts", bufs=1))
        sb = ctx.enter_context(tc.tile_pool(name="sb", bufs=2))
        psum = ctx.enter_context(tc.tile_pool(name="ps", bufs=1, space="PSUM"))
        dram = ctx.enter_context(tc.tile_pool(name="dr", bufs=1, space="DRAM"))

        def cload(ap, name, rows=None):
            shape = list(ap.shape) if rows is None else [rows, ap.shape[1]]
            t = cpool.tile(shape, ap.dtype, name=name, tag=name)
            nc.sync.dma_start(out=t[:], in_=ap if rows is None else ap[:rows, :])
            return t

        ident = cload(ident_i, "ident")
        rcg = cload(rcg_i, "rcg")
        wt = {}
        for l, (ci, co) in enumerate(dims):
            kc = -(-ci // 128)
            for side in ("wl", "wr"):
                for k in range(kc):
                    nm = f"{side}{l}k{k}"
                    t = cpool.tile([128, co], BF16, name=nm, tag=nm)
                    nc.sync.dma_start(
                        out=t[:], in_=w_i[f"{side}{l}"][k * 128:(k + 1) * 128, :])
                    wt[nm] = t
            for nm0 in ("brb", "bib", "attb"):
                wt[f"{nm0}{l}"] = cload(w_i[f"{nm0}{l}"], f"{nm0}{l}")
        fc1c, fc2c = [], []
        for k in range(KH):
            t = cpool.tile([128, nhid], F32, name=f"fc1k{k}", tag=f"fc1k{k}")
            nc.sync.dma_start(out=t[:], in_=fc1_i[k * 128:(k + 1) * 128, :])
            fc1c.append(t)
            t = cpool.tile([128, nout], F32, name=f"fc2k{k}", tag=f"fc2k{k}")
            nc.sync.dma_start(out=t[:], in_=fc2_i[k * 128:(k + 1) * 128, :])
            fc2c.append(t)
        b1b = cload(b1b_i, "b1b")
        b2b = cload(b2b_i, "b2b")

        # persistent DRAM buffers; AllGather outputs are distinct per layer
        # (a fast core's AG for layer l+1 may write a slow core's output
        # buffer while it still reads layer l's), and Shared for perf.
        xlf_space = "Shared" if n_cores > 4 else "Local"
        xlf = [dram.tile([NFULL, cpads[l]], BF16, name=f"xlf{l}", tag=f"xlf{l}",
                         addr_space=xlf_space) for l in range(n_layers)]
        xl_loc = [dram.tile([NMAXP, cpads[l]], BF16, name=f"xlloc{l}",
                            tag=f"xlloc{l}") for l in range(n_layers)]
        hbuf = [dram.tile([NMAXP, dims[l][1]], BF16, name=f"h{l}", tag=f"h{l}")
                for l in range(n_layers)]

        reg_cache = {}

        def nreg(v):
            if v not in reg_cache:
                reg_cache[v] = nc.gpsimd.to_reg(v)
            return reg_cache[v]

        # ---------------- layers ----------------
        # Emission is woven: edge-phase range r of layer l is followed by
        # node-phase range r of layer l+1 (or pooling range r after the last
        # layer), so every engine queue interleaves the two phases and the
        # AllGather for l+1 can fire right as edge phase l drains.
        wpc = cpool.tile([128, R * 128], BF16, name="wpc", tag="wpc")
        nc.sync.dma_start(out=wpc[:], in_=wp_i[:, :])

        xr_tiles = [
            sb.tile([128, R * dims[l][1]], BF16, name=f"xrsb{l}", tag="xrsb",
                    padded_shape=[128, R * COmax])
            for l in range(n_layers)]
        pg = psum.tile([128, nhid], F32, name="pg", tag="pg", bufs=1,
                       padded_shape=[128, max(nhid, COmax)])

        def emit_node(l, r):
            ci, co = dims[l]
            kc = -(-ci // 128)
            xr_sb = xr_tiles[l]
            hTs = []
            if l == 0:
                for k in range(kc):
                    hT = sb.tile([128, 128], BF16, name=f"hT{l}_{r}_{k}",
                                 tag=f"hT{k}")
                    nc.sync.dma_start(
                        out=hT[:],
                        in_=xT[k * 128:(k + 1) * 128, r * 128:(r + 1) * 128])
                    hTs.append(hT)
            else:
                hT = sb.tile([128, 128], BF16, name=f"hT{l}_{r}", tag="hT0",
                             padded_shape=[128, 128])
                nc.sync.dma_start(
                    out=hT[:ci, :],
                    in_=hbuf[l - 1][r * 128:(r + 1) * 128, :],
                    transpose=True)
                hTs.append(hT)
            krows = [128] * kc if l == 0 else [ci]
            pxl = psum.tile([128, co], F32, name=f"pxl{l}_{r}", tag="pnl",
                            bufs=1, padded_shape=[128, COmax])
            pxr = psum.tile([128, co], F32, name=f"pxr{l}_{r}", tag="pnr",
                            bufs=1, padded_shape=[128, COmax])
            for k in range(kc):
                nc.tensor.matmul(out=pxl[:], lhsT=hTs[k][:krows[k], :],
                                 rhs=wt[f"wl{l}k{k}"][:krows[k], :],
                                 start=(k == 0), stop=(k == kc - 1))
            for k in range(kc):
                nc.tensor.matmul(out=pxr[:], lhsT=hTs[k][:krows[k], :],
                                 rhs=wt[f"wr{l}k{k}"][:krows[k], :],
                                 start=(k == 0), stop=(k == kc - 1))
            xls = sb.tile([128, co], BF16, name=f"xls{l}_{r}", tag="xls",
                          padded_shape=[128, COmax])
            nc.scalar.activation(out=xls[:], in_=pxl[:], func=ACTF.Copy)
            nc.vector.tensor_tensor(out=xr_sb[:, r * co:(r + 1) * co],
                                    in0=pxr[:], in1=wt[f"brb{l}"][:],
                                    op=OP.add)
            nc.sync.dma_start(out=xl_loc[l][r * 128:(r + 1) * 128, :co],
                              in_=xls[:])

        def emit_pool(r):
            h3t = sb.tile([128, nhid], BF16, name=f"h3t{r}", tag="h3t")
            nc.sync.dma_start(out=h3t[:], in_=hbuf[-1][r * 128:(r + 1) * 128, :])
            nc.tensor.matmul(out=pg[:], lhsT=wpc[:, r * 128:(r + 1) * 128],
                             rhs=h3t[:], start=(r == 0), stop=(r == R - 1))

        def emit_ag(l):
            nc.gpsimd.collective_compute(
                "AllGather", OP.bypass, replica_groups=rg,
                ins=[xl_loc[l][:, :]], outs=[xlf[l][:, :]])

        offs = {}

        def emit_edge(l, r):
            ci, co = dims[l]
            cp = cpads[l]
            tlo, thi, tt = TLO[r], THI[r], T[r]
            olo, ohi, toff = offs.get(l, (0, 0, 0))
            offs[l] = (olo + 8 * tlo, ohi + 8 * thi, toff + tt)
            if tt == 0:
                return
            xr_sb = xr_tiles[l]
            gxl = sb.tile([128, tt * cp], BF16, name=f"gxl{l}_{r}", tag="gxl",
                          padded_shape=[128, Tmax * CPmax])
            g3l = gxl[:].rearrange("p (t c) -> p t c", c=cp)
            GCH = 8  # HW limit: <= 1024 idxs per dma_gather

            def chunked_gather(g3, t0, tn, src_ap, ix):
                for c0 in range(0, tn, GCH):
                    cn = min(GCH, tn - c0)
                    nc.gpsimd.dma_gather(
                        out_ap=g3[:, t0 + c0:t0 + c0 + cn, :],
                        in_ap=src_ap,
                        idxs_ap=ix[:, 8 * c0:8 * (c0 + cn)],
                        num_idxs=128 * cn, num_idxs_reg=nreg(128 * cn),
                        elem_size=cp)

            if tlo:
                ixlo = sb.tile([128, 8 * tlo], I16, name=f"ixlo{l}_{r}",
                               tag="ixlo", padded_shape=[128, 8 * Tmax])
                nc.sync.dma_start(out=ixlo[:],
                                  in_=idx_lo[:, olo:olo + 8 * tlo])
                chunked_gather(g3l, 0, tlo, xlf[l][0:min(split, NFULL), :],
                               ixlo)
            if thi:
                ixhi = sb.tile([128, 8 * thi], I16, name=f"ixhi{l}_{r}",
                               tag="ixhi", padded_shape=[128, 8 * Tmax])
                nc.sync.dma_start(out=ixhi[:],
                                  in_=idx_hi[:, ohi:ohi + 8 * thi])
                chunked_gather(g3l, tlo, thi, xlf[l][split:NFULL, :], ixhi)

            # one-hot tiles for this range
            w1 = sb.tile([128, tt * 128], BF16, name=f"w1{l}_{r}", tag="w1",
                         padded_shape=[128, Tmax * 128])
            nc.sync.dma_start(out=w1[:],
                              in_=w1_i[:, toff * 128:(toff + tt) * 128])
            w1t = sb.tile([128, tt * 128], BF16, name=f"w1t{l}_{r}",
                          tag="w1t", padded_shape=[128, Tmax * 128])
            nc.sync.dma_start(out=w1t[:],
                              in_=w1t_i[:, toff * 128:(toff + tt) * 128])

            u = g3l[:, :, :co]
            # vbuf holds m -> leaky_relu(m)*att -> [alpha-unnorm xl | s]
            vb = sb.tile([128, tt * (co + 1)], BF16, name=f"vb{l}_{r}",
                         tag="vb", padded_shape=[128, Tmax * (COmax + 1)])
            v3 = vb[:].rearrange("p (t c) -> p t c", c=co + 1)
            mv = v3[:, :, :co]
            # xr rows per edge via PE one-hot gather into a chunked PSUM
            # region (matmul outs may not cross the 2KB bank boundary, so
            # the per-tile stride is 128 f32 for co=128 and 512 for co=300)
            cps = 128 if co <= 128 else 512
            TCH = (3 * 512) // cps  # 6KB region = 3 banks
            for c0 in range(0, tt, TCH):
                cn = min(TCH, tt - c0)
                pxr_g = psum.tile([128, TCH * cps], F32,
                                  name=f"pxrg{l}_{r}_{c0}", tag="pxrg",
                                  bufs=1, padded_shape=[128, 3 * 512])
                x3r = pxr_g[:].rearrange("p (t c) -> p t c", c=cps)
                for t in range(c0, c0 + cn):
                    nc.tensor.matmul(out=x3r[:, t - c0, :co],
                                     lhsT=w1t[:, t * 128:(t + 1) * 128],
                                     rhs=xr_sb[:, r * co:(r + 1) * co],
                                     start=True, stop=True)
                # m = xl[src] + xr[dst]
                nc.vector.tensor_tensor(out=mv[:, c0:c0 + cn, :],
                                        in0=x3r[:, :cn, :co],
                                        in1=u[:, c0:c0 + cn, :], op=OP.add)
            # leaky_relu on the Scalar engine, in place
            nc.scalar.activation(out=mv, in_=mv, func=ACTF.Lrelu, alpha=SLOPE)
            att3 = wt[f"attb{l}"][:, None, :].to_broadcast([128, tt, co])
            nc.vector.tensor_tensor(out=mv, in0=mv, in1=att3, op=OP.mult)
            logits = sb.tile([128, tt], F32, name=f"lg{l}_{r}", tag="lg",
                             padded_shape=[128, Tmax])
            nc.vector.tensor_reduce(out=logits[:], in_=mv, axis=AX.X,
                                    op=OP.add)
            nc.scalar.activation(out=v3[:, :, co], in_=logits[:],
                                 func=ACTF.Exp)
            sbc = v3[:, :, co:co + 1].to_broadcast([128, tt, co])
            nc.vector.tensor_tensor(out=mv, in0=u, in1=sbc, op=OP.mult)

            pvd = psum.tile([128, co + 1], F32, name=f"pvd{l}_{r}",
                            tag="pvd", bufs=2,
                            padded_shape=[128, COmax + 1])
            for t in range(tt):
                nc.tensor.matmul(out=pvd[:],
                                 lhsT=w1[:, t * 128:(t + 1) * 128],
                                 rhs=v3[:, t, :],
                                 start=(t == 0), stop=(t == tt - 1))
            den = sb.tile([128, 1], F32, name=f"den{l}_{r}", tag="den")
            nc.vector.tensor_scalar_max(out=den[:], in0=pvd[:, co:co + 1],
                                        scalar1=1e-30)
            rden = sb.tile([128, 1], F32, name=f"rden{l}_{r}", tag="rden")
            nc.vector.reciprocal(out=rden[:], in_=den[:])
            hsb = sb.tile([128, co], BF16, name=f"hsb{l}_{r}", tag="hsb",
                          padded_shape=[128, COmax])
            nc.vector.scalar_tensor_tensor(
                out=hsb[:], in0=pvd[:, :co], scalar=rden[:, :],
                in1=wt[f"bib{l}"][:], op0=OP.mult, op1=OP.add)
            hsb2 = sb.tile([128, co], BF16, name=f"hsb2{l}_{r}", tag="hsb2",
                           padded_shape=[128, COmax])
            nc.scalar.activation(out=hsb2[:], in_=hsb[:], func=ACTF.Relu)
            nc.sync.dma_start(out=hbuf[l][r * 128:(r + 1) * 128, :],
                              in_=hsb2[:])

        for r in range(R):
            emit_node(0, r)
        emit_ag(0)
        for l in range(n_layers):
            for r in range(R):
                emit_edge(l, r)
                if l + 1 < n_layers:
                    emit_node(l + 1, r)
                else:
                    emit_pool(r)
            if l + 1 < n_layers:
                emit_ag(l + 1)

        gsb = sb.tile([128, nhid], F32, name="gsb", tag="gsb")
        nc.vector.tensor_scalar_mul(out=gsb[:], in0=pg[:], scalar1=rcg[:, :])

        # ---------------- head ----------------
        gTs = []
        for k in range(KH):
            cw = min(128, nhid - k * 128)
            ptk = psum.tile([cw, 128], F32, name=f"ptg{k}", tag="pnl", bufs=1,
                            padded_shape=[128, max(nhid, COmax)])
            nc.tensor.transpose(out=ptk[:], in_=gsb[:, k * 128:k * 128 + cw],
                                identity=ident[:])
            gT = sb.tile([128, gpc], F32, name=f"gT{k}", tag=f"gT{k}")
            nc.vector.tensor_copy(out=gT[:cw, :], in_=ptk[:cw, :gpc])
            gTs.append((gT, cw))
        pz = psum.tile([gpc, nhid], F32, name="pz", tag="pnl", bufs=1,
                       padded_shape=[128, max(nhid, COmax)])
        for k in range(KH):
            gT, cw = gTs[k]
            nc.tensor.matmul(out=pz[:], lhsT=gT[:cw, :], rhs=fc1c[k][:cw, :],
                             start=(k == 0), stop=(k == KH - 1))
        zsb = sb.tile([gpc, nhid], F32, name="zsb", tag="zsb")
        nc.vector.tensor_tensor(out=zsb[:], in0=pz[:], in1=b1b[:gpc, :],
                                op=OP.add)
        zsb2 = sb.tile([gpc, nhid], F32, name="zsb2", tag="zsb2")
        nc.scalar.activation(out=zsb2[:], in_=zsb[:], func=ACTF.Relu)
        zTs = []
        for k in range(KH):
            cw = min(128, nhid - k * 128)
            ptk = psum.tile([cw, gpc], F32, name=f"ptz{k}", tag="pnl", bufs=1,
                            padded_shape=[128, max(nhid, COmax)])
            nc.tensor.transpose(out=ptk[:], in_=zsb2[:, k * 128:k * 128 + cw],
                                identity=ident[:gpc, :gpc])
            zT = sb.tile([128, gpc], F32, name=f"zT{k}", tag=f"zT{k}")
            nc.vector.tensor_copy(out=zT[:cw, :], in_=ptk[:cw, :])
            zTs.append((zT, cw))
        nsplit = -(-nout // 512)
        nw = nout // nsplit
        osb = sb.tile([gpc, nout], F32, name="osb", tag="osb")
        for j in range(nsplit):
            po = psum.tile([gpc, nw], F32, name=f"po{j}", tag="pnr", bufs=1,
                           padded_shape=[128, max(nw, COmax)])
            for k in range(KH):
                zT, cw = zTs[k]
                nc.tensor.matmul(out=po[:], lhsT=zT[:cw, :],
                                 rhs=fc2c[k][:cw, j * nw:(j + 1) * nw],
                                 start=(k == 0), stop=(k == KH - 1))
            nc.vector.tensor_tensor(out=osb[:, j * nw:(j + 1) * nw], in0=po[:],
                                    in1=b2b[:gpc, j * nw:(j + 1) * nw],
                                    op=OP.add)
        nc.sync.dma_start(out=out_t[:, :], in_=osb[:])

    nc.compile()
    return nc


# ----------------------------------------------------------------------------
# Entry point
# ----------------------------------------------------------------------------

def _augment(inputs):
    inputs = dict(inputs)
    inputs["_dims"] = [(300, 128), (128, 128), (128, 128), (128, 300)]
    inputs["_nhid"] = 300
    inputs["_nout"] = 768
    inputs["_n_graphs"] = 256
    return inputs


def run(inputs, trace=False, n_cores=8):
    inputs = _augment(inputs)
    meta, in_maps = preprocess(inputs, n_cores=n_cores)
    nc = build_nc(meta)
    res = run_bass_kernel_spmd(nc, in_maps, core_ids=list(range(n_cores)),
                               trace=trace)
    out = np.concatenate([r["out"] for r in res.results], axis=0)
    return out, res


def kernel(**inputs):
    out, _ = run(inputs, trace=False)
    return out


# revision 12
# speedup vs baseline: 1.0689x; 1.0689x over previous
"""GATv2 AttentionEncoder kernel for Trainium2 (8 NeuronCores, Bass/Tile).

Strategy (sharding_hint: shard by graph):
  - 256 graphs -> 8 cores x 32 graphs. Each core owns a contiguous,
    graph-aligned node slice, padded to NMAXP rows (multiple of 128).
  - Per layer: node-phase matmuls (xl = h@Wl+bl, xr = h@Wr+br) run on the
    local slice in bf16; xl is AllGathered (src edges reference any node),
    xr stays resident in SBUF (edges are bucketed by dst core/range).
  - Edge phase: edges sorted by dst 128-node range. Host precomputes, per
    128-edge tile, the dst one-hot W1 [edge, dst] and its transpose W1T
    [dst, edge] (bf16, zero rows for padding slots). Per range: dma_gather
    xl rows (int16 idx, lo/hi window split for the 32k limit); xr rows come
    from a PE matmul with lhsT=W1T (no SWDGE gather); all per-edge
    elementwise work (leaky_relu, att-dot, exp, alpha-scaling) runs as
    whole-range batched DVE/ACT ops; the scatter is a PE matmul with
    lhsT=W1 and rhs=[alpha-scaled xl | exp(logit)] producing the weighted
    sum and the softmax denominator in one accumulation group.
  - Pooling is a host-precomputed one-hot matmul + 1/cnt post-scale; the
    MLP head runs per-core on its 32 graphs in fp32; host concatenates.
"""

import sys

sys.path.insert(0, "/opt/trn_rl_repo")

import contextlib

import ml_dtypes
import numpy as np

import concourse.bass as bass
import concourse.bacc as bacc
import concourse.mybir as mybir
import concourse.tile as tile
from concourse.bass_utils import run_bass_kernel_spmd

F32 = mybir.dt.float32
BF16 = mybir.dt.bfloat16
I16 = mybir.dt.int16
AX = mybir.AxisListType
OP = mybir.AluOpType
ACTF = mybir.ActivationFunctionType

SLOPE = 0.2
BF = ml_dtypes.bfloat16


# ----------------------------------------------------------------------------
# Host-side preprocessing
# ----------------------------------------------------------------------------

def _wrap_idx(arr):
    """[n] int array (n % 16 == 0) -> [128, n/16] int16, slot i at
    [i%16, i//16], replicated 8x across partition groups of 16."""
    n = len(arr)
    w = np.ascontiguousarray(arr.reshape(n // 16, 16).T).astype(np.int16)
    return np.tile(w, (8, 1))


def preprocess(inputs, n_cores=8, split=32768):
    x = np.asarray(inputs["x"], np.float32)
    ei = np.asarray(inputs["edge_index"], np.int64)
    batch = np.asarray(inputs["batch"], np.int64)
    N, n_in = x.shape
    G = inputs["_n_graphs"]
    gpc = G // n_cores
    dims = inputs["_dims"]
    ebytes = 2
    # row byte-stride of xl must be a multiple of 256B
    cpads = [(co * ebytes + 255) // 256 * 256 // ebytes for (_, co) in dims]

    cnt = np.bincount(batch, minlength=G)
    gs = np.add.reduceat(cnt, np.arange(0, G, gpc))  # nodes per core
    bounds = np.concatenate([[0], np.cumsum(gs)]).astype(np.int64)
    NMAXP = int((gs.max() + 127) // 128 * 128)
    R = NMAXP // 128
    NFULL = n_cores * NMAXP
    assert NFULL <= split * 2, (NFULL, split)

    # remap node ids into the padded global layout
    newid = np.empty(N, np.int64)
    for r in range(n_cores):
        n0, n1 = bounds[r], bounds[r + 1]
        newid[n0:n1] = NMAXP * r + np.arange(n1 - n0)

    loops = np.arange(N, dtype=np.int64)
    src = newid[np.concatenate([ei[0], loops])]
    dst = newid[np.concatenate([ei[1], loops])]

    # bucket edges per (core, range, lo/hi); order within a bucket irrelevant
    core_of = dst // NMAXP
    dstl = dst - core_of * NMAXP
    rng_of = dstl // 128
    is_hi = (src >= split).astype(np.int64)
    key = (core_of * R + rng_of) * 2 + is_hi
    order = np.argsort(key, kind="stable")
    src_s, dstl_s, key_s = src[order], dstl[order], key[order]
    uniq, starts = np.unique(key_s, return_index=True)
    starts = list(starts) + [len(key_s)]
    lo_lists = [[None] * R for _ in range(n_cores)]
    hi_lists = [[None] * R for _ in range(n_cores)]
    for i, k in enumerate(uniq):
        e0, e1 = starts[i], starts[i + 1]
        c, rem = divmod(int(k), 2 * R)
        g, h = divmod(rem, 2)
        pair = (src_s[e0:e1], dstl_s[e0:e1])
        (hi_lists if h else lo_lists)[c][g] = pair

    empty = (np.zeros(0, np.int64), np.zeros(0, np.int64))
    TLO = np.zeros(R, np.int64)
    THI = np.zeros(R, np.int64)
    for g in range(R):
        for c in range(n_cores):
            lo = lo_lists[c][g] or empty
            hi = hi_lists[c][g] or empty
            TLO[g] = max(TLO[g], -(-len(lo[0]) // 128))
            THI[g] = max(THI[g], -(-len(hi[0]) // 128))
    T = TLO + THI
    sumT = int(T.sum())

    meta = dict(
        n_cores=n_cores, gpc=gpc, G=G, NMAXP=NMAXP, R=R, NFULL=NFULL,
        split=split, TLO=TLO.tolist(), THI=THI.tolist(), T=T.tolist(),
        dims=dims, cpads=cpads, n_in=n_in, sumT=sumT,
        nhid=inputs["_nhid"], nout=inputs["_nout"],
    )

    # ---- shared const arrays ----
    ident = np.eye(128, dtype=np.float32)

    def bc(v, w, dt=np.float32):  # broadcast a [w] vector to [128, w]
        return np.tile(np.asarray(v, dt).reshape(1, w), (128, 1))

    def padk(w, dt=np.float32):  # pad leading dim to a multiple of 128
        k = (-(-w.shape[0] // 128)) * 128
        out = np.zeros((k,) + w.shape[1:], dt)
        out[: w.shape[0]] = np.asarray(w, dt)
        return out

    consts = dict(ident=ident)
    for l, (ci, co) in enumerate(dims):
        consts[f"wl{l}"] = padk(inputs[f"Wl{l}"], BF)
        consts[f"wr{l}"] = padk(inputs[f"Wr{l}"], BF)
        bl = np.asarray(inputs[f"bl{l}"], np.float64)
        consts[f"brb{l}"] = bc(np.asarray(inputs[f"br{l}"], np.float64) + bl, co)
        consts[f"bib{l}"] = bc(np.asarray(inputs[f"bias{l}"], np.float64) + bl, co)
        consts[f"attb{l}"] = bc(inputs[f"att{l}"], co, BF)
    consts["fc1"] = padk(inputs["fc1_W"])
    consts["fc2"] = padk(inputs["fc2_W"])
    consts["b1b"] = bc(inputs["fc1_b"], meta["nhid"])
    consts["b2b"] = bc(inputs["fc2_b"], meta["nout"])

    rcnt = 1.0 / np.maximum(cnt, 1).astype(np.float64)
    KIN = -(-n_in // 128)

    in_maps = []
    for c in range(n_cores):
        n0, n1 = bounds[c], bounds[c + 1]
        nl = int(n1 - n0)
        xT = np.zeros((KIN * 128, NMAXP), BF)
        xT[:n_in, :nl] = x[n0:n1].T.astype(BF)
        # per-core one-hot tiles + gather idxs
        w1 = np.zeros((128, sumT * 128), BF)
        w1t = np.zeros((128, sumT * 128), BF)
        ilo, ihi = [], []
        toff = 0
        for g in range(R):
            lo = lo_lists[c][g] or empty
            hi = hi_lists[c][g] or empty
            nlo, nhi = 128 * int(TLO[g]), 128 * int(THI[g])
            sl = np.zeros(nlo, np.int64)
            sl[: len(lo[0])] = lo[0]
            sh = np.zeros(nhi, np.int64)
            sh[: len(hi[0])] = hi[0] - split
            if nlo:
                ilo.append(_wrap_idx(sl))
            if nhi:
                ihi.append(_wrap_idx(sh))
            # real slots: dst one-hots (pad slots stay all-zero)
            dd = np.concatenate([lo[1], hi[1] + 0]).astype(np.int64)
            slot = np.concatenate(
                [np.arange(len(lo[1])), nlo + np.arange(len(hi[1]))])
            j = dd - g * 128  # local dst within range, 0..127
            p = slot % 128
            t = toff + slot // 128
            w1[p, t * 128 + j] = 1.0
            w1t[j, t * 128 + p] = 1.0
            toff += int(T[g])
        # pooling one-hot [node-in-range, graph-slot] per range, bf16 exact
        wp = np.zeros((128, R * 128), BF)
        bloc = (batch[n0:n1] - c * gpc).astype(np.int64)
        nodes = np.arange(nl)
        wp[nodes % 128, (nodes // 128) * 128 + bloc] = 1.0
        rc = np.zeros(128, np.float32)
        rc[:gpc] = rcnt[c * gpc:(c + 1) * gpc]
        m = dict(
            xT=xT, w1=w1, w1t=w1t, wp=wp, rcg=rc.reshape(128, 1),
            idx_lo=np.concatenate(ilo, 1) if ilo else np.zeros((128, 1), np.int16),
            idx_hi=np.concatenate(ihi, 1) if ihi else np.zeros((128, 1), np.int16),
        )
        m.update(consts)
        in_maps.append({k: np.ascontiguousarray(v) for k, v in m.items()})

    return meta, in_maps


# ----------------------------------------------------------------------------
# Bass program
# ----------------------------------------------------------------------------

def build_nc(meta):
    n_cores = meta["n_cores"]
    NMAXP, R, NFULL = meta["NMAXP"], meta["R"], meta["NFULL"]
    split = meta["split"]
    TLO, THI, T = meta["TLO"], meta["THI"], meta["T"]
    dims, cpads = meta["dims"], meta["cpads"]
    n_in, nhid, nout, gpc = meta["n_in"], meta["nhid"], meta["nout"], meta["gpc"]
    sumT = meta["sumT"]
    KIN = -(-n_in // 128)
    KH = -(-nhid // 128)
    n_layers = len(dims)
    Tmax = max(T)
    CPmax = max(cpads)
    COmax = max(co for _, co in dims)
    rg = [list(range(n_cores))]

    nc = bacc.Bacc(trn_type="TRN2", num_devices=n_cores)

    def inp(name, shape, dtype=F32):
        return nc.dram_tensor(name, list(shape), dtype, kind="ExternalInput").ap()

    xT = inp("xT", [KIN * 128, NMAXP], BF16)
    idx_lo = inp("idx_lo", [128, max(8 * sum(TLO), 1)], I16)
    idx_hi = inp("idx_hi", [128, max(8 * sum(THI), 1)], I16)
    w1_i = inp("w1", [128, sumT * 128], BF16)
    w1t_i = inp("w1t", [128, sumT * 128], BF16)
    wp_i = inp("wp", [128, R * 128], BF16)
    rcg_i = inp("rcg", [128, 1])
    ident_i = inp("ident", [128, 128])
    w_i = {}
    for l, (ci, co) in enumerate(dims):
        kc = -(-ci // 128)
        w_i[f"wl{l}"] = inp(f"wl{l}", [kc * 128, co], BF16)
        w_i[f"wr{l}"] = inp(f"wr{l}", [kc * 128, co], BF16)
        for nm in ("brb", "bib"):
            w_i[f"{nm}{l}"] = inp(f"{nm}{l}", [128, co])
        w_i[f"attb{l}"] = inp(f"attb{l}", [128, co], BF16)
    fc1_i = inp("fc1", [KH * 128, nhid])
    fc2_i = inp("fc2", [KH * 128, nout])
    b1b_i = inp("b1b", [128, nhid])
    b2b_i = inp("b2b", [128, nout])
    out_t = nc.dram_tensor("out", [gpc, nout], F32, kind="ExternalOutput").ap()

    with tile.TileContext(nc) as tc, contextlib.ExitStack() as ctx:
        cpool = ctx.enter_context(tc.tile_pool(name="consts", bufs=1))
        sb = ctx.enter_context(tc.tile_pool(name="sb", bufs=2))
        psum = ctx.enter_context(tc.tile_pool(name="ps", bufs=1, space="PSUM"))
        dram = ctx.enter_context(tc.tile_pool(name="dr", bufs=1, space="DRAM"))

        def cload(ap, name, rows=None):
            shape = list(ap.shape) if rows is None else [rows, ap.shape[1]]
            t = cpool.tile(shape, ap.dtype, name=name, tag=name)
            nc.sync.dma_start(out=t[:], in_=ap if rows is None else ap[:rows, :])
            return t

        ident = cload(ident_i, "ident")
        rcg = cload(rcg_i, "rcg")
        wt = {}
        for l, (ci, co) in enumerate(dims):
            kc = -(-ci // 128)
            for side in ("wl", "wr"):
                for k in range(kc):
                    nm = f"{side}{l}k{k}"
                    t = cpool.tile([128, co], BF16, name=nm, tag=nm)
                    nc.sync.dma_start(
                        out=t[:], in_=w_i[f"{side}{l}"][k * 128:(k + 1) * 128, :])
                    wt[nm] = t
            for nm0 in ("brb", "bib", "attb"):
                wt[f"{nm0}{l}"] = cload(w_i[f"{nm0}{l}"], f"{nm0}{l}")
        fc1c, fc2c = [], []
        for k in range(KH):
            t = cpool.tile([128, nhid], F32, name=f"fc1k{k}", tag=f"fc1k{k}")
            nc.sync.dma_start(out=t[:], in_=fc1_i[k * 128:(k + 1) * 128, :])
            fc1c.append(t)
            t = cpool.tile([128, nout], F32, name=f"fc2k{k}", tag=f"fc2k{k}")
            nc.sync.dma_start(out=t[:], in_=fc2_i[k * 128:(k + 1) * 128, :])
            fc2c.append(t)
        b1b = cload(b1b_i, "b1b")
        b2b = cload(b2b_i, "b2b")

        # persistent DRAM buffers; AllGather outputs are distinct per layer
        # (a fast core's AG for layer l+1 may write a slow core's output
        # buffer while it still reads layer l's), and Shared for perf.
        xlf_space = "Shared" if n_cores > 4 else "Local"
        xlf = [dram.tile([NFULL, cpads[l]], BF16, name=f"xlf{l}", tag=f"xlf{l}",
                         addr_space=xlf_space) for l in range(n_layers)]
        xl_loc = [dram.tile([NMAXP, cpads[l]], BF16, name=f"xlloc{l}",
                            tag=f"xlloc{l}") for l in range(n_layers)]
        hbuf = [dram.tile([NMAXP, dims[l][1]], BF16, name=f"h{l}", tag=f"h{l}")
                for l in range(n_layers)]

        reg_cache = {}

        def nreg(v):
            if v not in reg_cache:
                reg_cache[v] = nc.gpsimd.to_reg(v)
            return reg_cache[v]

        # ---------------- layers ----------------
        # Emission is woven: edge-phase range r of layer l is followed by
        # node-phase range r of layer l+1 (or pooling range r after the last
        # layer), so every engine queue interleaves the two phases and the
        # AllGather for l+1 can fire right as edge phase l drains.
        wpc = cpool.tile([128, R * 128], BF16, name="wpc", tag="wpc")
        nc.sync.dma_start(out=wpc[:], in_=wp_i[:, :])

        xr_tiles = [
            sb.tile([128, R * dims[l][1]], BF16, name=f"xrsb{l}", tag="xrsb",
                    padded_shape=[128, R * COmax])
            for l in range(n_layers)]
        pg = psum.tile([128, nhid], F32, name="pg", tag="pg", bufs=1,
                       padded_shape=[128, max(nhid, COmax)])

        def emit_node(l, r):
            ci, co = dims[l]
            kc = -(-ci // 128)
            xr_sb = xr_tiles[l]
            hTs = []
            if l == 0:
                for k in range(kc):
                    hT = sb.tile([128, 128], BF16, name=f"hT{l}_{r}_{k}",
                                 tag=f"hT{k}")
                    nc.sync.dma_start(
                        out=hT[:],
                        in_=xT[k * 128:(k + 1) * 128, r * 128:(r + 1) * 128])
                    hTs.append(hT)
            else:
                hT = sb.tile([128, 128], BF16, name=f"hT{l}_{r}", tag="hT0",
                             padded_shape=[128, 128])
                nc.sync.dma_start(
                    out=hT[:ci, :],
                    in_=hbuf[l - 1][r * 128:(r + 1) * 128, :],
                    transpose=True)
                hTs.append(hT)
            krows = [128] * kc if l == 0 else [ci]
            pxl = psum.tile([128, co], F32, name=f"pxl{l}_{r}", tag="pnl",
                            bufs=1, padded_shape=[128, COmax])
            pxr = psum.tile([128, co], F32, name=f"pxr{l}_{r}", tag="pnr",
                            bufs=1, padded_shape=[128, COmax])
            for k in range(kc):
                nc.tensor.matmul(out=pxl[:], lhsT=hTs[k][:krows[k], :],
                                 rhs=wt[f"wl{l}k{k}"][:krows[k], :],
                                 start=(k == 0), stop=(k == kc - 1))
            for k in range(kc):
                nc.tensor.matmul(out=pxr[:], lhsT=hTs[k][:krows[k], :],
                                 rhs=wt[f"wr{l}k{k}"][:krows[k], :],
                                 start=(k == 0), stop=(k == kc - 1))
            xls = sb.tile([128, co], BF16, name=f"xls{l}_{r}", tag="xls",
                          padded_shape=[128, COmax])
            nc.scalar.activation(out=xls[:], in_=pxl[:], func=ACTF.Copy)
            nc.vector.tensor_tensor(out=xr_sb[:, r * co:(r + 1) * co],
                                    in0=pxr[:], in1=wt[f"brb{l}"][:],
                                    op=OP.add)
            nc.sync.dma_start(out=xl_loc[l][r * 128:(r + 1) * 128, :co],
                              in_=xls[:])

        def emit_pool(r):
            h3t = sb.tile([128, nhid], BF16, name=f"h3t{r}", tag="h3t")
            nc.sync.dma_start(out=h3t[:], in_=hbuf[-1][r * 128:(r + 1) * 128, :])
            nc.tensor.matmul(out=pg[:], lhsT=wpc[:, r * 128:(r + 1) * 128],
                             rhs=h3t[:], start=(r == 0), stop=(r == R - 1))

        def emit_ag(l):
            nc.gpsimd.collective_compute(
                "AllGather", OP.bypass, replica_groups=rg,
                ins=[xl_loc[l][:, :]], outs=[xlf[l][:, :]])

        offs = {}

        def emit_edge(l, r):
            ci, co = dims[l]
            cp = cpads[l]
            tlo, thi, tt = TLO[r], THI[r], T[r]
            olo, ohi, toff = offs.get(l, (0, 0, 0))
            offs[l] = (olo + 8 * tlo, ohi + 8 * thi, toff + tt)
            if tt == 0:
                return
            xr_sb = xr_tiles[l]
            gxl = sb.tile([128, tt * cp], BF16, name=f"gxl{l}_{r}", tag="gxl",
                          padded_shape=[128, Tmax * CPmax])
            g3l = gxl[:].rearrange("p (t c) -> p t c", c=cp)
            GCH = 8  # HW limit: <= 1024 idxs per dma_gather

            def chunked_gather(g3, t0, tn, src_ap, ix):
                for c0 in range(0, tn, GCH):
                    cn = min(GCH, tn - c0)
                    nc.gpsimd.dma_gather(
                        out_ap=g3[:, t0 + c0:t0 + c0 + cn, :],
                        in_ap=src_ap,
                        idxs_ap=ix[:, 8 * c0:8 * (c0 + cn)],
                        num_idxs=128 * cn, num_idxs_reg=nreg(128 * cn),
                        elem_size=cp)

            if tlo:
                ixlo = sb.tile([128, 8 * tlo], I16, name=f"ixlo{l}_{r}",
                               tag="ixlo", padded_shape=[128, 8 * Tmax])
                nc.sync.dma_start(out=ixlo[:],
                                  in_=idx_lo[:, olo:olo + 8 * tlo])
                chunked_gather(g3l, 0, tlo, xlf[l][0:min(split, NFULL), :],
                               ixlo)
            if thi:
                ixhi = sb.tile([128, 8 * thi], I16, name=f"ixhi{l}_{r}",
                               tag="ixhi", padded_shape=[128, 8 * Tmax])
                nc.sync.dma_start(out=ixhi[:],
                                  in_=idx_hi[:, ohi:ohi + 8 * thi])
                chunked_gather(g3l, tlo, thi, xlf[l][split:NFULL, :], ixhi)

            # one-hot tiles for this range
            w1 = sb.tile([128, tt * 128], BF16, name=f"w1{l}_{r}", tag="w1",
                         padded_shape=[128, Tmax * 128])
            nc.sync.dma_start(out=w1[:],
                              in_=w1_i[:, toff * 128:(toff + tt) * 128])
            w1t = sb.tile([128, tt * 128], BF16, name=f"w1t{l}_{r}",
                          tag="w1t", padded_shape=[128, Tmax * 128])
            nc.sync.dma_start(out=w1t[:],
                              in_=w1t_i[:, toff * 128:(toff + tt) * 128])

            u = g3l[:, :, :co]
            # vbuf holds m -> leaky_relu(m)*att -> [alpha-unnorm xl | s]
            vb = sb.tile([128, tt * (co + 1)], BF16, name=f"vb{l}_{r}",
                         tag="vb", padded_shape=[128, Tmax * (COmax + 1)])
            v3 = vb[:].rearrange("p (t c) -> p t c", c=co + 1)
            mv = v3[:, :, :co]
            # xr rows per edge via PE one-hot gather into a chunked PSUM
            # region (matmul outs may not cross the 2KB bank boundary, so
            # the per-tile stride is 128 f32 for co=128 and 512 for co=300)
            cps = 128 if co <= 128 else 512
            TCH = (3 * 512) // cps  # 6KB region = 3 banks
            for c0 in range(0, tt, TCH):
                cn = min(TCH, tt - c0)
                pxr_g = psum.tile([128, TCH * cps], F32,
                                  name=f"pxrg{l}_{r}_{c0}", tag="pxrg",
                                  bufs=1, padded_shape=[128, 3 * 512])
                x3r = pxr_g[:].rearrange("p (t c) -> p t c", c=cps)
                for t in range(c0, c0 + cn):
                    nc.tensor.matmul(out=x3r[:, t - c0, :co],
                                     lhsT=w1t[:, t * 128:(t + 1) * 128],
                                     rhs=xr_sb[:, r * co:(r + 1) * co],
                                     start=True, stop=True)
                # m = xl[src] + xr[dst]
                nc.vector.tensor_tensor(out=mv[:, c0:c0 + cn, :],
                                        in0=x3r[:, :cn, :co],
                                        in1=u[:, c0:c0 + cn, :], op=OP.add)
            # leaky_relu(m) = max(0.2*m, m), in place
            nc.vector.scalar_tensor_tensor(out=mv, in0=mv, scalar=SLOPE,
                                           in1=mv, op0=OP.mult, op1=OP.max)
            att3 = wt[f"attb{l}"][:, None, :].to_broadcast([128, tt, co])
            nc.vector.tensor_tensor(out=mv, in0=mv, in1=att3, op=OP.mult)
            logits = sb.tile([128, tt], F32, name=f"lg{l}_{r}", tag="lg",
                             padded_shape=[128, Tmax])
            nc.vector.tensor_reduce(out=logits[:], in_=mv, axis=AX.X,
                                    op=OP.add)
            nc.scalar.activation(out=v3[:, :, co], in_=logits[:],
                                 func=ACTF.Exp)
            sbc = v3[:, :, co:co + 1].to_broadcast([128, tt, co])
            nc.vector.tensor_tensor(out=mv, in0=u, in1=sbc, op=OP.mult)

            pvd = psum.tile([128, co + 1], F32, name=f"pvd{l}_{r}",
                            tag="pvd", bufs=2,
                            padded_shape=[128, COmax + 1])
            for t in range(tt):
                nc.tensor.matmul(out=pvd[:],
                                 lhsT=w1[:, t * 128:(t + 1) * 128],
                                 rhs=v3[:, t, :],
                                 start=(t == 0), stop=(t == tt - 1))
            den = sb.tile([128, 1], F32, name=f"den{l}_{r}", tag="den")
            nc.vector.tensor_scalar_max(out=den[:], in0=pvd[:, co:co + 1],
                                        scalar1=1e-30)
            rden = sb.tile([128, 1], F32, name=f"rden{l}_{r}", tag="rden")
            nc.vector.reciprocal(out=rden[:], in_=den[:])
            hsb = sb.tile([128, co], BF16, name=f"hsb{l}_{r}", tag="hsb",
                          padded_shape=[128, COmax])
            nc.vector.scalar_tensor_tensor(
                out=hsb[:], in0=pvd[:, :co], scalar=rden[:, :],
                in1=wt[f"bib{l}"][:], op0=OP.mult, op1=OP.add)
            hsb2 = sb.tile([128, co], BF16, name=f"hsb2{l}_{r}", tag="hsb2",
                           padded_shape=[128, COmax])
            nc.scalar.activation(out=hsb2[:], in_=hsb[:], func=ACTF.Relu)
            nc.sync.dma_start(out=hbuf[l][r * 128:(r + 1) * 128, :],
                              in_=hsb2[:])

        for r in range(R):
            emit_node(0, r)
        emit_ag(0)
        for l in range(n_layers):
            for r in range(R):
                emit_edge(l, r)
                # weave next layer's node phase one range behind, so its
                # hbuf-dependent loads never block this layer's edge work
                # at an engine queue head
                if l + 1 < n_layers and r >= 1:
                    emit_node(l + 1, r - 1)
            if l + 1 < n_layers:
                emit_node(l + 1, R - 1)
                emit_ag(l + 1)
            else:
                for r in range(R):
                    emit_pool(r)

        gsb = sb.tile([128, nhid], F32, name="gsb", tag="gsb")
        nc.vector.tensor_scalar_mul(out=gsb[:], in0=pg[:], scalar1=rcg[:, :])

        # ---------------- head ----------------
        gTs = []
        for k in range(KH):
            cw = min(128, nhid - k * 128)
            ptk = psum.tile([cw, 128], F32, name=f"ptg{k}", tag="pnl", bufs=1,
                            padded_shape=[128, max(nhid, COmax)])
            nc.tensor.transpose(out=ptk[:], in_=gsb[:, k * 128:k * 128 + cw],
                                identity=ident[:])
            gT = sb.tile([128, gpc], F32, name=f"gT{k}", tag=f"gT{k}")
            nc.vector.tensor_copy(out=gT[:cw, :], in_=ptk[:cw, :gpc])
            gTs.append((gT, cw))
        pz = psum.tile([gpc, nhid], F32, name="pz", tag="pnl", bufs=1,
                       padded_shape=[128, max(nhid, COmax)])
        for k in range(KH):
            gT, cw = gTs[k]
            nc.tensor.matmul(out=pz[:], lhsT=gT[:cw, :], rhs=fc1c[k][:cw, :],
                             start=(k == 0), stop=(k == KH - 1))
        zsb = sb.tile([gpc, nhid], F32, name="zsb", tag="zsb")
        nc.vector.tensor_tensor(out=zsb[:], in0=pz[:], in1=b1b[:gpc, :],
                                op=OP.add)
        zsb2 = sb.tile([gpc, nhid], F32, name="zsb2", tag="zsb2")
        nc.scalar.activation(out=zsb2[:], in_=zsb[:], func=ACTF.Relu)
        zTs = []
        for k in range(KH):
            cw = min(128, nhid - k * 128)
            ptk = psum.tile([cw, gpc], F32, name=f"ptz{k}", tag="pnl", bufs=1,
                            padded_shape=[128, max(nhid, COmax)])
            nc.tensor.transpose(out=ptk[:], in_=zsb2[:, k * 128:k * 128 + cw],
                                identity=ident[:gpc, :gpc])
            zT = sb.tile([128, gpc], F32, name=f"zT{k}", tag=f"zT{k}")
            nc.vector.tensor_copy(out=zT[:cw, :], in_=ptk[:cw, :])
            zTs.append((zT, cw))
        nsplit = -(-nout // 512)
        nw = nout // nsplit
        osb = sb.tile([gpc, nout], F32, name="osb", tag="osb")
        for j in range(nsplit):
            po = psum.tile([gpc, nw], F32, name=f"po{j}", tag="pnr", bufs=1,
                           padded_shape=[128, max(nw, COmax)])
            for k in range(KH):
                zT, cw = zTs[k]
                nc.tensor.matmul(out=po[:], lhsT=zT[:cw, :],
                                 rhs=fc2c[k][:cw, j * nw:(j + 1) * nw],
                                 start=(k == 0), stop=(k == KH - 1))
            nc.vector.tensor_tensor(out=osb[:, j * nw:(j + 1) * nw], in0=po[:],
                                    in1=b2b[:gpc, j * nw:(j + 1) * nw],
                                    op=OP.add)
        nc.sync.dma_start(out=out_t[:, :], in_=osb[:])

    nc.compile()
    return nc


# ----------------------------------------------------------------------------
# Entry point
# ----------------------------------------------------------------------------

def _augment(inputs):
    inputs = dict(inputs)
    inputs["_dims"] = [(300, 128), (128, 128), (128, 128), (128, 300)]
    inputs["_nhid"] = 300
    inputs["_nout"] = 768
    inputs["_n_graphs"] = 256
    return inputs


def run(inputs, trace=False, n_cores=8):
    inputs = _augment(inputs)
    meta, in_maps = preprocess(inputs, n_cores=n_cores)
    nc = build_nc(meta)
    res = run_bass_kernel_spmd(nc, in_maps, core_ids=list(range(n_cores)),
                               trace=trace)
    out = np.concatenate([r["out"] for r in res.results], axis=0)
    return out, res


def kernel(**inputs):
    out, _ = run(inputs, trace=False)
    return out
